# revision 1
# baseline (speedup 1.0000x reference)
"""Trainium2 Bass kernel for nn_CanadarmJacob (centroidal-dynamics jacobian).

Pure data-parallel over 8 NeuronCores: core c processes samples
[c*256:(c+1)*256] of the n_samples axis (x 128 horizon = 32768 flat
samples/core).  Per core the work is split into NBLK blocks of 128
(partitions) x F (free) samples; every per-sample scalar channel lives as a
strided view into sample-major SBUF tiles, so each graph node is one
vector-engine instruction over 128*F samples.

The math is an algebraically reduced form of the reference (validated to
~1e-5 rel):
  RP = C - P ;  MC = m_i*C ;  U[a,dd,i] = RP[a]*(MC[dd] | m_i)
  G = suffix_j(U)  ->  G[a,d,j], R[a,j] (dd=3 row)
  r = sum_i MC / M_tot - (0,0,beta)
  T[a,j] = sum_d G[a,d,j]*J[d,j] ;  trG, rR, rJ
  u = trG - beta*R_z - rR ;  v = beta*J_z + rJ
  H_theta = (DCUM + u)*J - T + v*R
  J_tw = J_j x R_j
  H_s = K r r^T + diag(C1 - K|r|^2) ;  sInv = -adj(H_s)/det
  bot = sInv @ H_theta ;  top = -J_tw/M_tot + r x bot
"""

import os
import sys

for _p in ("/opt/trn_rl_repo", "/root/.axon_site/_ro/trn_rl_repo"):
    if os.path.isdir(_p) and _p not in sys.path:
        sys.path.append(_p)

import numpy as np

import concourse.bass as bass
import concourse.tile as tile
from concourse import bacc, mybir
from concourse.bass_utils import run_bass_kernel_spmd

# ----------------------------------------------------------------- constants
N_SAMPLES, N_HORIZON = 2048, 128
N_CORES = 8
P = 128
F = 32  # samples per partition row per block
SPC = N_SAMPLES // N_CORES * N_HORIZON  # samples per core = 32768
NBLK = SPC // (P * F)  # 8

BASE_MASS, EEF_MASS = 100000.0, 243.66
MASS = np.array([105.98, 105.98, 314.98, 279.2, 105.98, 105.98, 243.66], np.float32)
DIAGS = np.array(
    [
        [12.19, 12.19, 3.061],
        [12.19, 12.19, 3.061],
        [15.41, 2094.71, 2103.19],
        [9.522, 1966.28, 1966.28],
        [8.305, 3.061, 8.0386],
        [12.13, 12.13, 3.061],
        [9.336, 44.41, 44.41],
    ],
    np.float32,
)
I0DIAG = np.array([69585.02, 69585.02, 66666.664], np.float32)

M_MAN = float(MASS.sum())
M_TOT = M_MAN + BASE_MASS + EEF_MASS
K = BASE_MASS + EEF_MASS
BETA = 6.65 * (243.66 / (100000.0 + 243.66))
DCUM = np.stack([DIAGS[j:].sum(0) for j in range(7)], axis=1)  # [a][j]
C1 = DIAGS.sum(0) + I0DIAG  # [a]

DT = mybir.dt.float32
ADD = mybir.AluOpType.add
SUB = mybir.AluOpType.subtract
MUL = mybir.AluOpType.mult


def _const_array() -> np.ndarray:
    cst = np.zeros((P, 45), np.float32)
    cst[:, 0:21] = np.broadcast_to(MASS[None, :], (3, 7)).reshape(21)[None, :]
    cst[:, 21:42] = DCUM.reshape(21)[None, :]
    cst[:, 42:45] = C1[None, :]
    return cst


def build_nc():
    nc = bacc.Bacc("TRN2")

    com_in = nc.dram_tensor("com", [NBLK, P, F * 21], DT, kind="ExternalInput")
    lnk_in = nc.dram_tensor("lnk", [NBLK, P, F, 144], DT, kind="ExternalInput")
    jac_in = nc.dram_tensor("jac", [NBLK, P, F * 42], DT, kind="ExternalInput")
    cst_in = nc.dram_tensor("cst", [P, 45], DT, kind="ExternalInput")
    out_d = nc.dram_tensor("out", [NBLK, P, F * 42], DT, kind="ExternalOutput")

    V = nc.vector
    G_ = nc.gpsimd
    A_ = nc.scalar
    X = mybir.AxisListType.X

    with tile.TileContext(nc) as tc:
        with (
            tc.tile_pool(name="cstp", bufs=1) as cstp,
            tc.tile_pool(name="iocl", bufs=2) as iocl,
            tc.tile_pool(name="iojo", bufs=3) as iojo,
            tc.tile_pool(name="wk", bufs=3) as wk,
        ):
            cst = cstp.tile([P, 45], DT, tag="cst")
            nc.scalar.dma_start(cst[:], cst_in[:])
            massc = (
                cst[:, 0:21]
                .rearrange("p (a i) -> p a i", a=3, i=7)
                .unsqueeze(1)
                .broadcast_to([P, F, 3, 7])
            )
            dcum_v = (
                cst[:, 21:42]
                .rearrange("p (a j) -> p a j", a=3, j=7)
                .unsqueeze(1)
                .broadcast_to([P, F, 3, 7])
            )
            c1_v = cst[:, 42:45].unsqueeze(1).broadcast_to([P, F, 3])

            def rv(t, groups, **kw):
                return t[:].rearrange(f"p (f {groups}) -> p f {groups}", f=F, **kw)

            def front(b, E=None):
                """DMA + all Pool-engine work for block b.  Pool(b) depends
                only on block b's DMAs — it never waits on DVE."""
                st = {}
                E = E or G_
                comt = iocl.tile([P, F * 21], DT, tag="comt")
                lnkt = iocl.tile([P, F * 108], DT, tag="lnkt")
                jact = iojo.tile([P, F * 42], DT, tag="jact")
                nc.scalar.dma_start(comt[:], com_in[b])
                lnkv = lnkt[:].rearrange("p (f e) -> p f e", f=F, e=108)
                half = F // 2
                nc.sync.dma_start(lnkv[:, 0:half, :], lnk_in[b, :, 0:half, 0:108])
                nc.scalar.dma_start(lnkv[:, half:F, :], lnk_in[b, :, half:F, 0:108])
                nc.scalar.dma_start(jact[:], jac_in[b])

                comv = rv(comt, "a i", a=3, i=7)
                lall = rv(lnkt, "a q i", a=3, q=4, i=9)
                posv = lall[:, :, 0:3, 3, 0:7]
                jacv = rv(jact, "d j", d=6, j=7)
                j3 = jacv[:, :, 0:3, :]
                st["jact"], st["j3"] = jact, j3

                mc = wk.tile([P, F * 21], DT, tag="mc")
                mcv = rv(mc, "a i", a=3, i=7)
                G_.tensor_mul(mcv, comv, massc)

                # rpre[a] = sum_i MC[a][i]  (unscaled; /M_tot folded downstream)
                y9 = wk.tile([P, F * 9], DT, tag="y9")
                y9v = rv(y9, "a c", a=3, c=3)
                G_.tensor_add(y9v, mcv[:, :, :, 0:3], mcv[:, :, :, 3:6])
                rt = wk.tile([P, F * 3], DT, tag="rt")
                rtv = rv(rt, "a", a=3)
                G_.tensor_add(rtv, y9v[:, :, :, 0], y9v[:, :, :, 1])
                G_.tensor_add(rtv, rtv, y9v[:, :, :, 2])
                G_.tensor_add(rtv, rtv, mcv[:, :, :, 6])
                st["rtv"] = rtv

                # rj'[j] = sum_a rpre[a] * J[a][j]
                r_bj = rtv.unsqueeze(3).broadcast_to([P, F, 3, 7])
                rjp = wk.tile([P, F * 21], DT, tag="rjp")
                rjpv = rv(rjp, "a j", a=3, j=7)
                G_.tensor_mul(rjpv, r_bj, j3)
                rj = wk.tile([P, F * 7], DT, tag="rj")
                rjv = rv(rj, "j", j=7)
                G_.tensor_add(rjv, rjpv[:, :, 0, :], rjpv[:, :, 1, :])
                G_.tensor_add(rjv, rjv, rjpv[:, :, 2, :])

                rp = wk.tile([P, F * 21], DT, tag="rp")
                rpv = rv(rp, "a i", a=3, i=7)
                E.tensor_sub(rpv, comv, posv)

                ut = wk.tile([P, F * 84], DT, tag="ut")
                utv = rv(ut, "a dd i", a=3, dd=4, i=7)
                for a in range(3):
                    rp_b = rpv[:, :, a : a + 1, :].broadcast_to([P, F, 3, 7])
                    E.tensor_mul(utv[:, :, a, 0:3, :], rp_b, mcv)
                E.tensor_mul(utv[:, :, :, 3, :], rpv, massc)

                # suffix sums over last index (j = 5..0), in place on ut
                gtv = rv(ut, "a dd j", a=3, dd=4, j=7)
                for j in range(5, -1, -1):
                    E.tensor_add(
                        gtv[:, :, :, :, j], gtv[:, :, :, :, j], gtv[:, :, :, :, j + 1]
                    )
                gd = gtv[:, :, :, 0:3, :]
                rsuf = gtv[:, :, :, 3, :]
                st["gd"], st["rsuf"] = gd, rsuf

                trg = wk.tile([P, F * 7], DT, tag="trg")
                trgv = rv(trg, "j", j=7)
                E.tensor_add(trgv, gd[:, :, 0, 0, :], gd[:, :, 1, 1, :])
                E.tensor_add(trgv, trgv, gd[:, :, 2, 2, :])
                st["trgv"] = trgv

                # VR' = rj'_b * R
                vrv = rjpv  # rjp dead after tree; reuse for VR'
                v_b = rjv.unsqueeze(2).broadcast_to([P, F, 3, 7])
                G_.tensor_mul(vrv, v_b, rsuf)
                st["vrv"] = vrv



                st["mcv"] = mcv  # mc dead after rpre; reuse for H_theta
                st["rpv"] = rpv  # rp dead after U-products; reuse for RRp'/T
                return st

            def back(st, b, EJ=None):
                """All DVE work for block b (+ ACT bits + output DMA)."""
                EJ = EJ or V
                j3, gd, rsuf = st["j3"], st["gd"], st["rsuf"]
                rtv = st["rtv"]

                # rs = rpre/M_tot - (0,0,beta)
                rs = wk.tile([P, F * 3], DT, tag="rs")
                rsv = rv(rs, "a", a=3)
                A_.mul(rsv, st["rtv"], 1.0 / M_TOT)
                V.tensor_scalar_add(rsv[:, :, 2:3], rsv[:, :, 2:3], -BETA)

                # RRp' = rpre_b * R ; rr' tree ; u = trG - rr'/M_tot
                rrpv = st["rpv"]  # reuse rp tile
                r_bj = rtv.unsqueeze(3).broadcast_to([P, F, 3, 7])
                V.tensor_mul(rrpv, r_bj, rsuf)
                rr = wk.tile([P, F * 7], DT, tag="rr")
                rrv = rv(rr, "j", j=7)
                V.tensor_add(rrv, rrpv[:, :, 0, :], rrpv[:, :, 1, :])
                V.tensor_add(rrv, rrv, rrpv[:, :, 2, :])
                ut7 = wk.tile([P, F * 7], DT, tag="ut7")
                ut7v = rv(ut7, "j", j=7)
                V.scalar_tensor_tensor(ut7v, rrv, -1.0 / M_TOT, st["trgv"], MUL, ADD)

                a1 = wk.tile([P, F * 21], DT, tag="a1")
                a1v = rv(a1, "a j", a=3, j=7)
                u_b = ut7v.unsqueeze(2).broadcast_to([P, F, 3, 7])
                V.tensor_add(a1v, u_b, dcum_v)

                # PROD1 overwrites gd in place; T tree into rrp (rp tile, now dead)
                for a in range(3):
                    V.tensor_mul(gd[:, :, a, :, :], gd[:, :, a, :, :], j3)
                ttv = rrpv
                V.tensor_add(ttv, gd[:, :, :, 0, :], gd[:, :, :, 1, :])
                V.tensor_add(ttv, ttv, gd[:, :, :, 2, :])

                # H_theta = A1*J - T + VR'/M_tot   (into mc tile)
                hthv = st["mcv"]
                V.tensor_mul(hthv, a1v, j3)
                V.tensor_sub(hthv, hthv, ttv)
                V.scalar_tensor_tensor(hthv, st["vrv"], 1.0 / M_TOT, hthv, MUL, ADD)

                # J_tw[a] = J[a1]*R[a2] - J[a2]*R[a1]
                jtw = wk.tile([P, F * 21], DT, tag="jtw")
                jtwv = rv(jtw, "a j", a=3, j=7)
                cx1 = wk.tile([P, F * 21], DT, tag="cx1")
                cx1v = rv(cx1, "a j", a=3, j=7)
                for a in range(3):
                    a1_, a2_ = (a + 1) % 3, (a + 2) % 3
                    EJ.tensor_mul(jtwv[:, :, a, :], j3[:, :, a1_, :], rsuf[:, :, a2_, :])
                    EJ.tensor_mul(cx1v[:, :, a, :], j3[:, :, a2_, :], rsuf[:, :, a1_, :])
                EJ.tensor_sub(jtwv, jtwv, cx1v)

                # --------------------------------------- H_s + inverse
                rk = wk.tile([P, F * 3], DT, tag="rk")
                rkv = rv(rk, "a", a=3)
                A_.mul(rkv, rsv, K)

                hs = wk.tile([P, F * 6], DT, tag="hs")
                hsv = rv(hs, "k", k=6)
                rk0 = rkv[:, :, 0:1].broadcast_to([P, F, 3])
                rk1 = rkv[:, :, 1:2].broadcast_to([P, F, 2])
                V.tensor_mul(hsv[:, :, 0:3], rk0, rsv[:, :, 0:3])
                V.tensor_mul(hsv[:, :, 3:5], rk1, rsv[:, :, 1:3])
                V.tensor_mul(hsv[:, :, 5], rkv[:, :, 2], rsv[:, :, 2])
                kr2 = wk.tile([P, F], DT, tag="kr2")
                kr2v = kr2[:]
                V.tensor_add(kr2v, hsv[:, :, 0], hsv[:, :, 3])
                V.tensor_add(kr2v, kr2v, hsv[:, :, 5])
                t3 = wk.tile([P, F * 3], DT, tag="t3")
                t3v = rv(t3, "a", a=3)
                V.tensor_tensor(
                    t3v, c1_v, kr2v.unsqueeze(2).broadcast_to([P, F, 3]), SUB
                )
                V.tensor_add(hsv[:, :, 0:4:3], hsv[:, :, 0:4:3], t3v[:, :, 0:2])
                V.tensor_add(hsv[:, :, 5], hsv[:, :, 5], t3v[:, :, 2])

                cof = wk.tile([P, F * 6], DT, tag="cof")
                cofv = rv(cof, "k", k=6)
                cp = [(3, 5), (2, 4), (1, 4), (0, 5), (1, 2), (0, 3)]
                cq = [(4, 4), (1, 5), (2, 3), (2, 2), (0, 4), (1, 1)]
                tmp6 = wk.tile([P, F * 6], DT, tag="tmp6")
                tmp6v = rv(tmp6, "k", k=6)
                for k in range(6):
                    V.tensor_mul(cofv[:, :, k], hsv[:, :, cp[k][0]], hsv[:, :, cp[k][1]])
                    V.tensor_mul(tmp6v[:, :, k], hsv[:, :, cq[k][0]], hsv[:, :, cq[k][1]])
                V.tensor_sub(cofv, cofv, tmp6v)

                detp = wk.tile([P, F * 3], DT, tag="detp")
                detpv = rv(detp, "a", a=3)
                V.tensor_mul(detpv, hsv[:, :, 0:3], cofv[:, :, 0:3])
                det = wk.tile([P, F], DT, tag="det")
                detv = det[:]
                V.tensor_add(detv, detpv[:, :, 0], detpv[:, :, 1])
                V.tensor_add(detv, detv, detpv[:, :, 2])
                rec = wk.tile([P, F], DT, tag="rec")
                recv = rec[:]
                V.reciprocal(recv, detv)
                sinv = wk.tile([P, F * 6], DT, tag="sinv")
                sinvv = rv(sinv, "k", k=6)
                V.scalar_tensor_tensor(
                    sinvv,
                    cofv,
                    -1.0,
                    recv.unsqueeze(2).broadcast_to([P, F, 6]),
                    MUL,
                    MUL,
                )

                # --------------------------------------- outputs
                outt = iojo.tile([P, F * 42], DT, tag="outt")
                outv = rv(outt, "row j", row=6, j=7)

                # bot[a] = sum_d sInv[rows[a][d]] * Hth[d]; d=0 column of the
                # symmetric inverse is sinv[0:3] (contiguous) -> one 21F op
                bta = wk.tile([P, F * 21], DT, tag="bta")
                btav = rv(bta, "a j", a=3, j=7)
                btb = wk.tile([P, F * 21], DT, tag="btb")
                btbv = rv(btb, "a j", a=3, j=7)
                s_d0 = sinvv.unsqueeze(3)[:, :, 0:3, :].broadcast_to([P, F, 3, 7])
                h_d0 = hthv[:, :, 0:1, :].broadcast_to([P, F, 3, 7])
                V.tensor_mul(btav, s_d0, h_d0)
                for a, k in enumerate((1, 3, 4)):  # d=1 column: rows (1,3,4)
                    sk = sinvv[:, :, k].unsqueeze(2).broadcast_to([P, F, 7])
                    V.tensor_mul(btbv[:, :, a, :], sk, hthv[:, :, 1, :])
                V.tensor_add(btav, btav, btbv)
                for a, k in enumerate((2, 4, 5)):  # d=2 column: rows (2,4,5)
                    sk = sinvv[:, :, k].unsqueeze(2).broadcast_to([P, F, 7])
                    V.tensor_mul(btbv[:, :, a, :], sk, hthv[:, :, 2, :])
                V.tensor_add(outv[:, :, 3:6, :], btav, btbv)

                # top[a] = -Jtw[a]/M_tot + (r x bot)[a]
                ctbv = a1v  # a1 dead after hth mul
                ctcv = cx1v  # cx1 dead after jtw sub
                for a in range(3):
                    a1_, a2_ = (a + 1) % 3, (a + 2) % 3
                    r1 = rsv[:, :, a1_].unsqueeze(2).broadcast_to([P, F, 7])
                    r2 = rsv[:, :, a2_].unsqueeze(2).broadcast_to([P, F, 7])
                    EJ.tensor_mul(ctbv[:, :, a, :], r1, outv[:, :, 3 + a2_, :])
                    EJ.tensor_mul(ctcv[:, :, a, :], r2, outv[:, :, 3 + a1_, :])
                V.tensor_sub(ctbv, ctbv, ctcv)
                V.scalar_tensor_tensor(
                    outv[:, :, 0:3, :], jtwv, -1.0 / M_TOT, ctbv, MUL, ADD
                )

                nc.scalar.dma_start(out_d[b], outt[:])

            st_prev = None
            for b in range(NBLK):
                st = front(b, E=V if b == 0 else None)
                if st_prev is not None:
                    back(st_prev, b - 1, EJ=G_ if b - 1 >= NBLK - 2 else None)
                st_prev = st
            back(st_prev, NBLK - 1, EJ=G_)

    nc.compile()
    return nc


_NC_CACHE = None


def _get_nc():
    global _NC_CACHE
    if _NC_CACHE is None:
        _NC_CACHE = build_nc()
    return _NC_CACHE


def _shard_inputs(com_list, link_pose_list, jacobian):
    com = np.ascontiguousarray(np.asarray(com_list, np.float32))
    lnk = np.ascontiguousarray(np.asarray(link_pose_list, np.float32))
    jac = np.ascontiguousarray(np.asarray(jacobian, np.float32))
    npc = N_SAMPLES // N_CORES
    cst = _const_array()
    in_maps = []
    for c in range(N_CORES):
        sl = slice(c * npc, (c + 1) * npc)
        in_maps.append(
            {
                "com": com[sl].reshape(NBLK, P, F * 21),
                "lnk": lnk[sl].reshape(NBLK, P, F, 144),
                "jac": jac[sl].reshape(NBLK, P, F * 42),
                "cst": cst,
            }
        )
    return in_maps


def _gather(results):
    outs = [r["out"].reshape(-1, 6, 7) for r in results]
    full = np.concatenate(outs, axis=0)
    return full.reshape(N_SAMPLES, N_HORIZON, 6, 7).astype(np.float32)


def run(com_list, link_pose_list, jacobian, trace=False):
    nc = _get_nc()
    in_maps = _shard_inputs(com_list, link_pose_list, jacobian)
    res = run_bass_kernel_spmd(nc, in_maps, list(range(N_CORES)), trace=trace)
    return _gather(res.results), res


def kernel(com_list, link_pose_list, jacobian):
    out, _ = run(com_list, link_pose_list, jacobian)
    return out



# revision 11
# speedup vs baseline: 1.3889x; 1.3889x over previous
"""Trainium2 Bass kernel for nn_CanadarmJacob (centroidal-dynamics jacobian).

v2: fp16 compute in channel-major layout [P=128 partitions, ch, F samples],
data-parallel over 8 cores.  Host repacks inputs to the 63 used channels
(com 21, link-pos 21, jac rows0:3 21) as fp16; device computes the reduced
graph (validated vs reference in fp32 and fp16, rel ~1.6e-3):

  RP = C - P ; MC = C*m/S ; U = [RP_a*MC_d ; RP*m/M_tot]
  G,R = suffix_j(U) ; trg = tr(G) ; rj = sum_a rpre_a J ; VR = rj_b * R
  rr = sum_a rpre_a R ; u = trg - rr          (R pre-scaled by 1/M_tot)
  hth = (u+DCUM/S)*J - sum_d G*J + VR         (= true H_theta / S)
  jtw = J x R                                 (= true J_tw / M_tot)
  H_s^-1 via 1st-order Neumann (diag-dominant): q = DS/d,
  bot = -g + T2 - T3 with g = q*hth*(S/DS), T2/T3 the off-diag corrections
  top = (r x bot) - jtw

All tensor ops keep the sample dim innermost & packed -> DVE 2x fp16 mode;
Pool ops are emitted as scalar_tensor_tensor (cheaper gpsimd path), which
requires <=3D operands, so contiguous (ch,F) pairs are pre-merged.
"""

import os
import sys

for _p in ("/opt/trn_rl_repo", "/root/.axon_site/_ro/trn_rl_repo"):
    if os.path.isdir(_p) and _p not in sys.path:
        sys.path.append(_p)

import numpy as np

import concourse.bass as bass
import concourse.tile as tile
from concourse import bacc, mybir
from concourse.bass_utils import run_bass_kernel_spmd

# ----------------------------------------------------------------- constants
N_SAMPLES, N_HORIZON = 2048, 128
N_CORES = 8
P = 128
F = 64  # samples per partition row per block
SPC = N_SAMPLES // N_CORES * N_HORIZON  # samples per core = 32768
NBLK = SPC // (P * F)  # 4

BASE_MASS, EEF_MASS = 100000.0, 243.66
MASS = np.array([105.98, 105.98, 314.98, 279.2, 105.98, 105.98, 243.66], np.float32)
DIAGS = np.array(
    [
        [12.19, 12.19, 3.061],
        [12.19, 12.19, 3.061],
        [15.41, 2094.71, 2103.19],
        [9.522, 1966.28, 1966.28],
        [8.305, 3.061, 8.0386],
        [12.13, 12.13, 3.061],
        [9.336, 44.41, 44.41],
    ],
    np.float32,
)
I0DIAG = np.array([69585.02, 69585.02, 66666.664], np.float32)

M_TOT = float(MASS.sum()) + BASE_MASS + EEF_MASS
K = BASE_MASS + EEF_MASS
BETA = 6.65 * (243.66 / (100000.0 + 243.66))
DCUM = np.stack([DIAGS[j:].sum(0) for j in range(7)], axis=1)  # [a][j]
C1 = DIAGS.sum(0) + I0DIAG  # [a]

S = 64.0   # mass scale for MC/G/hth/...
RS = 16.0  # rE = RS * r
DS = 64.0  # dd = d / DS

DT = mybir.dt.float16
ADD = mybir.AluOpType.add
SUB = mybir.AluOpType.subtract
MUL = mybir.AluOpType.mult

NCCH = 41  # const channels: mp 7, mpm 7, dcum 21, c1 3, bvec 3


def _const_array() -> np.ndarray:
    cst = np.zeros((P, NCCH, F), np.float32)
    cst[:, 0:7, :] = (MASS / S)[None, :, None]
    cst[:, 7:14, :] = (MASS / M_TOT)[None, :, None]
    cst[:, 14:35, :] = (DCUM / S).reshape(21)[None, :, None]
    cst[:, 35:38, :] = (C1 / DS)[None, :, None]
    cst[:, 40, :] = RS * BETA
    return cst.astype(np.float16)


def build_nc():
    nc = bacc.Bacc("TRN2")

    in_d = nc.dram_tensor("inp", [NBLK, P, 63 * F], DT, kind="ExternalInput")
    cst_d = nc.dram_tensor("cst", [P, NCCH * F], DT, kind="ExternalInput")
    out_d = nc.dram_tensor("out", [NBLK, P, 42 * F], DT, kind="ExternalOutput")

    V = nc.vector
    G_ = nc.gpsimd
    A_ = nc.scalar

    # Pool elementwise ops (neuronxcc rejects TensorScalarPtr on Pool).
    def gm(out, a, b):
        G_.tensor_mul(out, a, b)

    def ga(out, a, b):
        G_.tensor_add(out, a, b)

    def gs(out, a, b):
        G_.tensor_sub(out, a, b)

    with tile.TileContext(nc) as tc:
        with (
            nc.allow_low_precision(reason="fp16 graph validated vs fp32 reference"),
            tc.tile_pool(name="cstp", bufs=1) as cstp,
            tc.tile_pool(name="ioin", bufs=2) as ioin,
            tc.tile_pool(name="ioout", bufs=2) as ioout,
            tc.tile_pool(name="wk", bufs=2) as wk,
        ):
            cst = cstp.tile([P, NCCH * F], DT, tag="cst")
            nc.scalar.dma_start(cst[:], cst_d[:])
            cf = cst[:]
            mp_b = (
                cf[:, 0 : 7 * F].rearrange("p (o x) -> p o x", o=1, x=7 * F)
                .broadcast_to([P, 3, 7 * F])
            )
            mpm_b = (
                cf[:, 7 * F : 14 * F].rearrange("p (o x) -> p o x", o=1, x=7 * F)
                .broadcast_to([P, 3, 7 * F])
            )
            dcum_v = cf[:, 14 * F : 35 * F].rearrange("p (a x) -> p a x", a=3, x=7 * F)
            c1_v = cf[:, 35 * F : 38 * F].rearrange("p (a f) -> p a f", a=3, f=F)
            bvec_f = cf[:, 38 * F : 41 * F]  # [P, 3F] = (0,0,RS*BETA)
            zero_b = (
                cf[:, 38 * F : 39 * F].rearrange("p (o f) -> p o f", o=1, f=F)
                .broadcast_to([P, 3, F])
            )

            def t(ch, tag):
                return wk.tile([P, ch * F], DT, tag=tag, name=tag)[:]

            def front(b):
                st = {}
                int_ = ioin.tile([P, 63 * F], DT, tag="int")
                nc.scalar.dma_start(int_[:], in_d[b])
                iv = int_[:]
                com_f = iv[:, 0 : 21 * F]
                pos_f = iv[:, 21 * F : 42 * F]
                jac_f = iv[:, 42 * F : 63 * F]
                com3 = com_f.rearrange("p (a x) -> p a x", a=3, x=7 * F)
                jac3 = jac_f.rearrange("p (a x) -> p a x", a=3, x=7 * F)
                jac4 = jac_f.rearrange("p (a j f) -> p a j f", a=3, j=7, f=F)
                st["jac3"], st["jac4"], st["jac_f"] = jac3, jac4, jac_f

                # RP = C - P (Pool, 2D)
                rp = t(21, "rp")
                gs(rp, com_f, pos_f)
                rp3 = rp.rearrange("p (a x) -> p a x", a=3, x=7 * F)
                # MC = C * m/S (DVE)
                mc = t(21, "mc")
                mc3 = mc.rearrange("p (a x) -> p a x", a=3, x=7 * F)
                V.tensor_mul(mc3, com3, mp_b)

                # U[a,dd,i]: dd 0:3 = RP_a * MC_d (DVE) ; dd=3 = RP*m/M (Pool)
                ut = t(84, "ut")
                u4 = ut.rearrange("p (a d x) -> p a d x", a=3, d=4, x=7 * F)
                rp_b = rp3.unsqueeze(2).broadcast_to([P, 3, 3, 7 * F])
                mc_b = mc3.unsqueeze(1).broadcast_to([P, 3, 3, 7 * F])
                V.tensor_mul(u4[:, :, 0:3], rp_b, mc_b)
                gm(u4[:, :, 3], rp3, mpm_b)

                # rpre tree (DVE)
                mc_ai = mc.rearrange("p (a i f) -> p a i f", a=3, i=7, f=F)
                y9 = t(9, "y9").rearrange("p (a c f) -> p a c f", a=3, c=3, f=F)
                V.tensor_add(y9, mc_ai[:, :, 0:3], mc_ai[:, :, 3:6])
                rpre = t(3, "rpre").rearrange("p (a f) -> p a f", a=3, f=F)
                V.tensor_add(rpre, y9[:, :, 0], y9[:, :, 1])
                V.tensor_add(rpre, rpre, y9[:, :, 2])
                V.tensor_add(rpre, rpre, mc_ai[:, :, 6])
                st["rpre"] = rpre

                # suffix sums over links (in place, DVE)
                gt = ut.rearrange("p (c j f) -> p c j f", c=12, j=7, f=F)
                for j in range(5, -1, -1):
                    V.tensor_add(gt[:, :, j], gt[:, :, j], gt[:, :, j + 1])
                st["u4"] = u4
                rsuf3 = u4[:, :, 3]  # R[a,j] * 1/M_tot (as [P,3,7F])
                st["rsuf3"] = rsuf3

                trg = t(7, "trg")
                V.tensor_add(trg, u4[:, 0, 0], u4[:, 1, 1])
                V.tensor_add(trg, trg, u4[:, 2, 2])
                st["trg"] = trg

                # rj = sum_a rpre_a J[a,:] (DVE mul + Pool tree)
                rjp = t(21, "rjp")
                rjp4 = rjp.rearrange("p (a j f) -> p a j f", a=3, j=7, f=F)
                rpre_bj = rpre.unsqueeze(2).broadcast_to([P, 3, 7, F])
                V.tensor_mul(rjp4, rpre_bj, jac4)
                rj = t(7, "rj")
                ga(rj, rjp[:, 0 : 7 * F], rjp[:, 7 * F : 14 * F])
                ga(rj, rj, rjp[:, 14 * F : 21 * F])

                # VR = rj_b * R (Pool, 3D)
                vr = t(21, "vr")
                vr3 = vr.rearrange("p (a x) -> p a x", a=3, x=7 * F)
                rj_b = (
                    rj.rearrange("p (o x) -> p o x", o=1, x=7 * F)
                    .broadcast_to([P, 3, 7 * F])
                )
                gm(vr3, rj_b, rsuf3)
                st["vr"] = vr

                # rr = sum_a rpre_a R[a,:] (DVE mul + Pool tree)
                rrp = t(21, "rrp")
                rrp4 = rrp.rearrange("p (a j f) -> p a j f", a=3, j=7, f=F)
                rsuf4 = u4.rearrange("p a d x -> p a d x")[:, :, 3].rearrange(
                    "p a (j f) -> p a j f", j=7, f=F
                )
                V.tensor_mul(rrp4, rpre_bj, rsuf4)
                rr = t(7, "rr")
                ga(rr, rrp[:, 0 : 7 * F], rrp[:, 7 * F : 14 * F])
                ga(rr, rr, rrp[:, 14 * F : 21 * F])
                st["rr"] = rr
                return st

            def back(st, b):
                jac3, jac4, jac_f = st["jac3"], st["jac4"], st["jac_f"]
                u4, rsuf3 = st["u4"], st["rsuf3"]
                rpre, trg, rr = st["rpre"], st["trg"], st["rr"]
                vr = st["vr"]

                outt = ioout.tile([P, 42 * F], DT, tag="outt")
                ov = outt[:]
                top_f = ov[:, 0 : 21 * F]
                bot_f = ov[:, 21 * F : 42 * F]
                bot4 = bot_f.rearrange("p (a j f) -> p a j f", a=3, j=7, f=F)

                # u = trg - rr (DVE, rr pre-scaled by 1/M via mpm)
                u7 = t(7, "u7")
                V.tensor_sub(u7, trg, rr)
                # A1 = u_b + DCUM/S (DVE)
                a1 = t(21, "a1")
                a1_3 = a1.rearrange("p (a x) -> p a x", a=3, x=7 * F)
                u_b = (
                    u7.rearrange("p (o x) -> p o x", o=1, x=7 * F)
                    .broadcast_to([P, 3, 7 * F])
                )
                V.tensor_add(a1_3, u_b, dcum_v)

                # P1 = G * J_d (in-place on gd), T tree (DVE)
                gd = u4[:, :, 0:3]
                jac_bd = jac3.unsqueeze(1).broadcast_to([P, 3, 3, 7 * F])
                V.tensor_mul(gd, gd, jac_bd)
                t1 = t(21, "t1")
                t1_3 = t1.rearrange("p (a x) -> p a x", a=3, x=7 * F)
                V.tensor_add(t1_3, u4[:, :, 0], u4[:, :, 1])
                V.tensor_add(t1_3, t1_3, u4[:, :, 2])

                # hth = A1*J - T + VR (DVE, flat)
                hth = t(21, "hth")
                V.tensor_mul(hth, a1, jac_f)
                V.tensor_sub(hth, hth, t1)
                V.tensor_add(hth, hth, st["vr"])
                hth4 = hth.rearrange("p (a j f) -> p a j f", a=3, j=7, f=F)

                # jtw = J x R (Pool; slices are contiguous 7F runs)
                jtw = t(21, "jtw")
                cx1 = t(21, "cx1")
                jtw3 = jtw.rearrange("p (a x) -> p a x", a=3, x=7 * F)
                cx13 = cx1.rearrange("p (a x) -> p a x", a=3, x=7 * F)
                for a in range(3):
                    a1_, a2_ = (a + 1) % 3, (a + 2) % 3
                    gm(jtw3[:, a], jac3[:, a1_], rsuf3[:, a2_])
                    gm(cx13[:, a], jac3[:, a2_], rsuf3[:, a1_])
                gs(jtw, jtw, cx1)

                # rE = RS*r = rpre*(RS*S/M) - (0,0,RS*BETA) (DVE stt)
                rpre_f = rpre.rearrange("p a f -> p (a f)")
                rE = t(3, "rE")
                V.scalar_tensor_tensor(rE, rpre_f, RS * S / M_TOT, bvec_f, MUL, SUB)
                rE3 = rE.rearrange("p (a f) -> p a f", a=3, f=F)
                # p2 = rE^2, s2 = sum p2 (DVE)
                p2 = t(3, "p2")
                V.tensor_mul(p2, rE, rE)
                s2 = t(1, "s2")
                V.tensor_add(s2, p2[:, 0:F], p2[:, F : 2 * F])
                V.tensor_add(s2, s2, p2[:, 2 * F : 3 * F])
                # dd = (C1 - K|r|^2)/DS + (K/DS) r_a^2 ; q = 1/dd
                KS = float(K / (RS * RS * DS))
                t3 = t(3, "t3")
                t3_3 = t3.rearrange("p (a f) -> p a f", a=3, f=F)
                s2_b = (
                    s2.rearrange("p (o f) -> p o f", o=1, f=F).broadcast_to([P, 3, F])
                )
                V.scalar_tensor_tensor(t3_3, s2_b, -KS, c1_v, MUL, ADD)
                dd = t(3, "dd")
                V.scalar_tensor_tensor(dd, p2, KS, t3, MUL, ADD)
                qq = t(3, "qq")
                V.reciprocal(qq, dd)
                qq3 = qq.rearrange("p (a f) -> p a f", a=3, f=F)

                # g = q_b * hth  (S/DS == 1)
                g = t(21, "g")
                g4 = g.rearrange("p (a j f) -> p a j f", a=3, j=7, f=F)
                qq_b = qq3.unsqueeze(2).broadcast_to([P, 3, 7, F])
                V.tensor_mul(g4, qq_b, hth4)
                # w = sum_a rE_a g_a (DVE mul + Pool tree)
                u1 = t(21, "u1")
                u1_4 = u1.rearrange("p (a j f) -> p a j f", a=3, j=7, f=F)
                rE_b = rE3.unsqueeze(2).broadcast_to([P, 3, 7, F])
                V.tensor_mul(u1_4, rE_b, g4)
                w7 = t(7, "w7")
                ga(w7, u1[:, 0 : 7 * F], u1[:, 7 * F : 14 * F])
                ga(w7, w7, u1[:, 14 * F : 21 * F])
                # v1 = (q*K/..)*rE ; T2 = v1_b * w_b (DVE)
                v1 = t(3, "v1")
                V.scalar_tensor_tensor(v1, qq, KS, rE, MUL, MUL)
                z1 = t(21, "z1")
                z1_4 = z1.rearrange("p (a j f) -> p a j f", a=3, j=7, f=F)
                v1_b = (
                    v1.rearrange("p (a f) -> p a f", a=3, f=F)
                    .unsqueeze(2)
                    .broadcast_to([P, 3, 7, F])
                )
                w_b = (
                    w7.rearrange("p (o x) -> p o x", o=1, x=7 * F)
                    .rearrange("p o (j f) -> p o j f", j=7, f=F)
                    .broadcast_to([P, 3, 7, F])
                )
                V.tensor_mul(z1_4, v1_b, w_b)
                # s3 = (p2*K/..)*q ; T3 = s3_b*g ; z1 -= T3 ; bot = z1-g
                s3 = t(3, "s3")
                V.scalar_tensor_tensor(s3, p2, KS, qq, MUL, MUL)
                t3t = t(21, "t3t")
                t3t4 = t3t.rearrange("p (a j f) -> p a j f", a=3, j=7, f=F)
                s3_b = (
                    s3.rearrange("p (a f) -> p a f", a=3, f=F)
                    .unsqueeze(2)
                    .broadcast_to([P, 3, 7, F])
                )
                V.tensor_mul(t3t4, s3_b, g4)
                V.tensor_sub(z1, z1, t3t)
                V.tensor_sub(bot_f, z1, g)

                # top = (r x bot) - jtw   (r = rE/RS)
                rQ = t(3, "rQ")
                rQ3 = rQ.rearrange("p (a f) -> p a f", a=3, f=F)
                V.scalar_tensor_tensor(rQ3, rE3, 1.0 / RS, zero_b, MUL, ADD)
                ctb = t(21, "ctb")
                ctc = t(21, "ctc")
                ctb3 = ctb.rearrange("p (a x) -> p a x", a=3, x=7 * F)
                ctc3 = ctc.rearrange("p (a x) -> p a x", a=3, x=7 * F)
                bot3 = bot_f.rearrange("p (a x) -> p a x", a=3, x=7 * F)
                for a in range(3):
                    a1_, a2_ = (a + 1) % 3, (a + 2) % 3
                    r1 = rQ3[:, a1_ : a1_ + 1, :].broadcast_to([P, 7, F])
                    r2 = rQ3[:, a2_ : a2_ + 1, :].broadcast_to([P, 7, F])
                    b2 = bot3[:, a2_].rearrange("p (j f) -> p j f", j=7, f=F)
                    b1 = bot3[:, a1_].rearrange("p (j f) -> p j f", j=7, f=F)
                    V.tensor_mul(ctb3[:, a].rearrange("p (j f) -> p j f", j=7, f=F), r1, b2)
                    V.tensor_mul(ctc3[:, a].rearrange("p (j f) -> p j f", j=7, f=F), r2, b1)
                gs(ctb, ctb, ctc)
                V.tensor_sub(top_f, ctb, jtw)

                nc.scalar.dma_start(out_d[b], outt[:])

            st_prev = None
            for b in range(NBLK):
                st = front(b)
                if st_prev is not None:
                    back(st_prev, b - 1)
                st_prev = st
            back(st_prev, NBLK - 1)

    nc.compile()
    return nc


_NC_CACHE = None


def _get_nc():
    global _NC_CACHE
    if _NC_CACHE is None:
        _NC_CACHE = build_nc()
    return _NC_CACHE


def _shard_inputs(com_list, link_pose_list, jacobian):
    com = np.asarray(com_list, np.float32).reshape(N_SAMPLES * N_HORIZON, 3, 7)
    lnk = np.asarray(link_pose_list, np.float32).reshape(
        N_SAMPLES * N_HORIZON, 4, 4, 9
    )
    jac = np.asarray(jacobian, np.float32).reshape(N_SAMPLES * N_HORIZON, 6, 7)
    pos = lnk[:, :3, 3, :7]  # (n,3,7)
    j3 = jac[:, :3, :]  # (n,3,7)

    # pack channels: [com 21, pos 21, jac 21] -> fp16 ch-major [NBLK,P,63,F]
    packed = np.concatenate(
        [com.reshape(-1, 21), pos.reshape(-1, 21), j3.reshape(-1, 21)], axis=1
    ).astype(np.float16)  # (n, 63)

    cst = _const_array().reshape(P, NCCH * F)
    in_maps = []
    for c in range(N_CORES):
        blk = packed[c * SPC : (c + 1) * SPC].reshape(NBLK, P, F, 63)
        blk = np.ascontiguousarray(blk.transpose(0, 1, 3, 2))  # [NBLK,P,63,F]
        in_maps.append({"inp": blk.reshape(NBLK, P, 63 * F), "cst": cst})
    return in_maps


def _gather(results):
    outs = []
    for r in results:
        o = np.asarray(r["out"], np.float32).reshape(NBLK, P, 42, F)
        o = o.transpose(0, 1, 3, 2).reshape(SPC, 6, 7)
        outs.append(o)
    full = np.concatenate(outs, axis=0)
    return np.ascontiguousarray(full.reshape(N_SAMPLES, N_HORIZON, 6, 7))


def run(com_list, link_pose_list, jacobian, trace=False):
    nc = _get_nc()
    in_maps = _shard_inputs(com_list, link_pose_list, jacobian)
    res = run_bass_kernel_spmd(nc, in_maps, list(range(N_CORES)), trace=trace)
    return _gather(res.results), res


def kernel(com_list, link_pose_list, jacobian):
    out, _ = run(com_list, link_pose_list, jacobian)
    return out


# revision 12
# speedup vs baseline: 1.4079x; 1.0137x over previous
"""Trainium2 Bass kernel for nn_CanadarmJacob (centroidal-dynamics jacobian).

v2: fp16 compute in channel-major layout [P=128 partitions, ch, F samples],
data-parallel over 8 cores.  Host repacks inputs to the 63 used channels
(com 21, link-pos 21, jac rows0:3 21) as fp16; device computes the reduced
graph (validated vs reference in fp32 and fp16, rel ~1.6e-3):

  RP = C - P ; MC = C*m/S ; U = [RP_a*MC_d ; RP*m/M_tot]
  G,R = suffix_j(U) ; trg = tr(G) ; rj = sum_a rpre_a J ; VR = rj_b * R
  rr = sum_a rpre_a R ; u = trg - rr          (R pre-scaled by 1/M_tot)
  hth = (u+DCUM/S)*J - sum_d G*J + VR         (= true H_theta / S)
  jtw = J x R                                 (= true J_tw / M_tot)
  H_s^-1 via 1st-order Neumann (diag-dominant): q = DS/d,
  bot = -g + T2 - T3 with g = q*hth*(S/DS), T2/T3 the off-diag corrections
  top = (r x bot) - jtw

All tensor ops keep the sample dim innermost & packed -> DVE 2x fp16 mode;
Pool ops are emitted as scalar_tensor_tensor (cheaper gpsimd path), which
requires <=3D operands, so contiguous (ch,F) pairs are pre-merged.
"""

import os
import sys

for _p in ("/opt/trn_rl_repo", "/root/.axon_site/_ro/trn_rl_repo"):
    if os.path.isdir(_p) and _p not in sys.path:
        sys.path.append(_p)

import numpy as np

import concourse.bass as bass
import concourse.tile as tile
from concourse import bacc, mybir
from concourse.bass_utils import run_bass_kernel_spmd

# ----------------------------------------------------------------- constants
N_SAMPLES, N_HORIZON = 2048, 128
N_CORES = 8
P = 128
F = 64  # samples per partition row per block
SPC = N_SAMPLES // N_CORES * N_HORIZON  # samples per core = 32768
NBLK = SPC // (P * F)  # 4

BASE_MASS, EEF_MASS = 100000.0, 243.66
MASS = np.array([105.98, 105.98, 314.98, 279.2, 105.98, 105.98, 243.66], np.float32)
DIAGS = np.array(
    [
        [12.19, 12.19, 3.061],
        [12.19, 12.19, 3.061],
        [15.41, 2094.71, 2103.19],
        [9.522, 1966.28, 1966.28],
        [8.305, 3.061, 8.0386],
        [12.13, 12.13, 3.061],
        [9.336, 44.41, 44.41],
    ],
    np.float32,
)
I0DIAG = np.array([69585.02, 69585.02, 66666.664], np.float32)

M_TOT = float(MASS.sum()) + BASE_MASS + EEF_MASS
K = BASE_MASS + EEF_MASS
BETA = 6.65 * (243.66 / (100000.0 + 243.66))
DCUM = np.stack([DIAGS[j:].sum(0) for j in range(7)], axis=1)  # [a][j]
C1 = DIAGS.sum(0) + I0DIAG  # [a]

S = 64.0   # mass scale for MC/G/hth/...
RS = 16.0  # rE = RS * r
DS = 64.0  # dd = d / DS

DT = mybir.dt.float16
ADD = mybir.AluOpType.add
SUB = mybir.AluOpType.subtract
MUL = mybir.AluOpType.mult

NCCH = 41  # const channels: mp 7, mpm 7, dcum 21, c1 3, bvec 3


def _const_array() -> np.ndarray:
    cst = np.zeros((P, NCCH, F), np.float32)
    cst[:, 0:7, :] = (MASS / S)[None, :, None]
    cst[:, 7:14, :] = (MASS / M_TOT)[None, :, None]
    cst[:, 14:35, :] = (DCUM / S).reshape(21)[None, :, None]
    cst[:, 35:38, :] = (C1 / DS)[None, :, None]
    cst[:, 40, :] = RS * BETA
    return cst.astype(np.float16)


def build_nc():
    nc = bacc.Bacc("TRN2")

    in_d = nc.dram_tensor("inp", [NBLK, P, 63 * F], DT, kind="ExternalInput")
    cst_d = nc.dram_tensor("cst", [P, NCCH * F], DT, kind="ExternalInput")
    out_d = nc.dram_tensor("out", [NBLK, P, 42 * F], DT, kind="ExternalOutput")

    V = nc.vector
    G_ = nc.gpsimd
    A_ = nc.scalar

    # Pool elementwise ops (neuronxcc rejects TensorScalarPtr on Pool).
    def gm(out, a, b):
        G_.tensor_mul(out, a, b)

    def ga(out, a, b):
        G_.tensor_add(out, a, b)

    def gs(out, a, b):
        G_.tensor_sub(out, a, b)

    with tile.TileContext(nc) as tc:
        with (
            nc.allow_low_precision(reason="fp16 graph validated vs fp32 reference"),
            tc.tile_pool(name="cstp", bufs=1) as cstp,
            tc.tile_pool(name="ioin", bufs=2) as ioin,
            tc.tile_pool(name="ioout", bufs=2) as ioout,
            tc.tile_pool(name="wk", bufs=2) as wk,
        ):
            cst = cstp.tile([P, NCCH * F], DT, tag="cst")
            nc.scalar.dma_start(cst[:], cst_d[:])
            cf = cst[:]
            mp_b = (
                cf[:, 0 : 7 * F].rearrange("p (o x) -> p o x", o=1, x=7 * F)
                .broadcast_to([P, 3, 7 * F])
            )
            mpm_b = (
                cf[:, 7 * F : 14 * F].rearrange("p (o x) -> p o x", o=1, x=7 * F)
                .broadcast_to([P, 3, 7 * F])
            )
            dcum_v = cf[:, 14 * F : 35 * F].rearrange("p (a x) -> p a x", a=3, x=7 * F)
            c1_v = cf[:, 35 * F : 38 * F].rearrange("p (a f) -> p a f", a=3, f=F)
            bvec_f = cf[:, 38 * F : 41 * F]  # [P, 3F] = (0,0,RS*BETA)
            zero_b = (
                cf[:, 38 * F : 39 * F].rearrange("p (o f) -> p o f", o=1, f=F)
                .broadcast_to([P, 3, F])
            )

            def t(ch, tag):
                return wk.tile([P, ch * F], DT, tag=tag, name=tag)[:]

            def front(b):
                st = {}
                int_ = ioin.tile([P, 63 * F], DT, tag="int")
                nc.sync.dma_start(int_[:], in_d[b])
                iv = int_[:]
                com_f = iv[:, 0 : 21 * F]
                pos_f = iv[:, 21 * F : 42 * F]
                jac_f = iv[:, 42 * F : 63 * F]
                com3 = com_f.rearrange("p (a x) -> p a x", a=3, x=7 * F)
                jac3 = jac_f.rearrange("p (a x) -> p a x", a=3, x=7 * F)
                jac4 = jac_f.rearrange("p (a j f) -> p a j f", a=3, j=7, f=F)
                st["jac3"], st["jac4"], st["jac_f"] = jac3, jac4, jac_f

                # RP = C - P (Pool, 2D)
                rp = t(21, "rp")
                gs(rp, com_f, pos_f)
                rp3 = rp.rearrange("p (a x) -> p a x", a=3, x=7 * F)
                # MC = C * m/S (DVE)
                mc = t(21, "mc")
                mc3 = mc.rearrange("p (a x) -> p a x", a=3, x=7 * F)
                V.tensor_mul(mc3, com3, mp_b)

                # U[a,dd,i]: dd 0:3 = RP_a * MC_d (DVE) ; dd=3 = RP*m/M (Pool)
                ut = t(84, "ut")
                u4 = ut.rearrange("p (a d x) -> p a d x", a=3, d=4, x=7 * F)
                rp_b = rp3.unsqueeze(2).broadcast_to([P, 3, 3, 7 * F])
                mc_b = mc3.unsqueeze(1).broadcast_to([P, 3, 3, 7 * F])
                V.tensor_mul(u4[:, :, 0:3], rp_b, mc_b)
                gm(u4[:, :, 3], rp3, mpm_b)

                # rpre tree (DVE)
                mc_ai = mc.rearrange("p (a i f) -> p a i f", a=3, i=7, f=F)
                y9 = t(9, "y9").rearrange("p (a c f) -> p a c f", a=3, c=3, f=F)
                V.tensor_add(y9, mc_ai[:, :, 0:3], mc_ai[:, :, 3:6])
                rpre = t(3, "rpre").rearrange("p (a f) -> p a f", a=3, f=F)
                V.tensor_add(rpre, y9[:, :, 0], y9[:, :, 1])
                V.tensor_add(rpre, rpre, y9[:, :, 2])
                V.tensor_add(rpre, rpre, mc_ai[:, :, 6])
                st["rpre"] = rpre

                # suffix sums over links (in place, DVE)
                gt = ut.rearrange("p (c j f) -> p c j f", c=12, j=7, f=F)
                for j in range(5, -1, -1):
                    V.tensor_add(gt[:, :, j], gt[:, :, j], gt[:, :, j + 1])
                st["u4"] = u4
                rsuf3 = u4[:, :, 3]  # R[a,j] * 1/M_tot (as [P,3,7F])
                st["rsuf3"] = rsuf3

                trg = t(7, "trg")
                V.tensor_add(trg, u4[:, 0, 0], u4[:, 1, 1])
                V.tensor_add(trg, trg, u4[:, 2, 2])
                st["trg"] = trg

                # rj = sum_a rpre_a J[a,:] (DVE mul + Pool tree)
                rjp = t(21, "rjp")
                rjp4 = rjp.rearrange("p (a j f) -> p a j f", a=3, j=7, f=F)
                rpre_bj = rpre.unsqueeze(2).broadcast_to([P, 3, 7, F])
                V.tensor_mul(rjp4, rpre_bj, jac4)
                rj = t(7, "rj")
                ga(rj, rjp[:, 0 : 7 * F], rjp[:, 7 * F : 14 * F])
                ga(rj, rj, rjp[:, 14 * F : 21 * F])

                # VR = rj_b * R (Pool, 3D)
                vr = t(21, "vr")
                vr3 = vr.rearrange("p (a x) -> p a x", a=3, x=7 * F)
                rj_b = (
                    rj.rearrange("p (o x) -> p o x", o=1, x=7 * F)
                    .broadcast_to([P, 3, 7 * F])
                )
                gm(vr3, rj_b, rsuf3)
                st["vr"] = vr

                # rr = sum_a rpre_a R[a,:] (DVE mul + Pool tree)
                rrp = t(21, "rrp")
                rrp4 = rrp.rearrange("p (a j f) -> p a j f", a=3, j=7, f=F)
                rsuf4 = u4.rearrange("p a d x -> p a d x")[:, :, 3].rearrange(
                    "p a (j f) -> p a j f", j=7, f=F
                )
                V.tensor_mul(rrp4, rpre_bj, rsuf4)
                rr = t(7, "rr")
                ga(rr, rrp[:, 0 : 7 * F], rrp[:, 7 * F : 14 * F])
                ga(rr, rr, rrp[:, 14 * F : 21 * F])
                st["rr"] = rr
                return st

            def back(st, b):
                jac3, jac4, jac_f = st["jac3"], st["jac4"], st["jac_f"]
                u4, rsuf3 = st["u4"], st["rsuf3"]
                rpre, trg, rr = st["rpre"], st["trg"], st["rr"]
                vr = st["vr"]

                outt = ioout.tile([P, 42 * F], DT, tag="outt")
                ov = outt[:]
                top_f = ov[:, 0 : 21 * F]
                bot_f = ov[:, 21 * F : 42 * F]
                bot4 = bot_f.rearrange("p (a j f) -> p a j f", a=3, j=7, f=F)

                # u = trg - rr (DVE, rr pre-scaled by 1/M via mpm)
                u7 = t(7, "u7")
                V.tensor_sub(u7, trg, rr)
                # A1 = u_b + DCUM/S (DVE)
                a1 = t(21, "a1")
                a1_3 = a1.rearrange("p (a x) -> p a x", a=3, x=7 * F)
                u_b = (
                    u7.rearrange("p (o x) -> p o x", o=1, x=7 * F)
                    .broadcast_to([P, 3, 7 * F])
                )
                V.tensor_add(a1_3, u_b, dcum_v)

                # P1 = G * J_d (in-place on gd), T tree (DVE)
                gd = u4[:, :, 0:3]
                jac_bd = jac3.unsqueeze(1).broadcast_to([P, 3, 3, 7 * F])
                V.tensor_mul(gd, gd, jac_bd)
                t1 = t(21, "t1")
                t1_3 = t1.rearrange("p (a x) -> p a x", a=3, x=7 * F)
                V.tensor_add(t1_3, u4[:, :, 0], u4[:, :, 1])
                V.tensor_add(t1_3, t1_3, u4[:, :, 2])

                # hth = A1*J - T + VR (DVE, flat)
                hth = t(21, "hth")
                V.tensor_mul(hth, a1, jac_f)
                V.tensor_sub(hth, hth, t1)
                V.tensor_add(hth, hth, st["vr"])
                hth4 = hth.rearrange("p (a j f) -> p a j f", a=3, j=7, f=F)

                # jtw = J x R (Pool; slices are contiguous 7F runs)
                jtw = t(21, "jtw")
                cx1 = t(21, "cx1")
                jtw3 = jtw.rearrange("p (a x) -> p a x", a=3, x=7 * F)
                cx13 = cx1.rearrange("p (a x) -> p a x", a=3, x=7 * F)
                for a in range(3):
                    a1_, a2_ = (a + 1) % 3, (a + 2) % 3
                    gm(jtw3[:, a], jac3[:, a1_], rsuf3[:, a2_])
                    gm(cx13[:, a], jac3[:, a2_], rsuf3[:, a1_])
                gs(jtw, jtw, cx1)

                # rE = RS*r = rpre*(RS*S/M) - (0,0,RS*BETA) (DVE stt)
                rpre_f = rpre.rearrange("p a f -> p (a f)")
                rE = t(3, "rE")
                V.scalar_tensor_tensor(rE, rpre_f, RS * S / M_TOT, bvec_f, MUL, SUB)
                rE3 = rE.rearrange("p (a f) -> p a f", a=3, f=F)
                # p2 = rE^2, s2 = sum p2 (DVE)
                p2 = t(3, "p2")
                V.tensor_mul(p2, rE, rE)
                s2 = t(1, "s2")
                V.tensor_add(s2, p2[:, 0:F], p2[:, F : 2 * F])
                V.tensor_add(s2, s2, p2[:, 2 * F : 3 * F])
                # dd = (C1 - K|r|^2)/DS + (K/DS) r_a^2 ; q = 1/dd
                KS = float(K / (RS * RS * DS))
                t3 = t(3, "t3")
                t3_3 = t3.rearrange("p (a f) -> p a f", a=3, f=F)
                s2_b = (
                    s2.rearrange("p (o f) -> p o f", o=1, f=F).broadcast_to([P, 3, F])
                )
                V.scalar_tensor_tensor(t3_3, s2_b, -KS, c1_v, MUL, ADD)
                dd = t(3, "dd")
                V.scalar_tensor_tensor(dd, p2, KS, t3, MUL, ADD)
                qq = t(3, "qq")
                V.reciprocal(qq, dd)
                qq3 = qq.rearrange("p (a f) -> p a f", a=3, f=F)

                # g = q_b * hth  (S/DS == 1)
                g = t(21, "g")
                g4 = g.rearrange("p (a j f) -> p a j f", a=3, j=7, f=F)
                qq_b = qq3.unsqueeze(2).broadcast_to([P, 3, 7, F])
                V.tensor_mul(g4, qq_b, hth4)
                # w = sum_a rE_a g_a (DVE mul + Pool tree)
                u1 = t(21, "u1")
                u1_4 = u1.rearrange("p (a j f) -> p a j f", a=3, j=7, f=F)
                rE_b = rE3.unsqueeze(2).broadcast_to([P, 3, 7, F])
                V.tensor_mul(u1_4, rE_b, g4)
                w7 = t(7, "w7")
                ga(w7, u1[:, 0 : 7 * F], u1[:, 7 * F : 14 * F])
                ga(w7, w7, u1[:, 14 * F : 21 * F])
                # v1 = (q*K/..)*rE ; T2 = v1_b * w_b (DVE)
                v1 = t(3, "v1")
                V.scalar_tensor_tensor(v1, qq, KS, rE, MUL, MUL)
                z1 = t(21, "z1")
                z1_4 = z1.rearrange("p (a j f) -> p a j f", a=3, j=7, f=F)
                v1_b = (
                    v1.rearrange("p (a f) -> p a f", a=3, f=F)
                    .unsqueeze(2)
                    .broadcast_to([P, 3, 7, F])
                )
                w_b = (
                    w7.rearrange("p (o x) -> p o x", o=1, x=7 * F)
                    .rearrange("p o (j f) -> p o j f", j=7, f=F)
                    .broadcast_to([P, 3, 7, F])
                )
                V.tensor_mul(z1_4, v1_b, w_b)
                # s3 = (p2*K/..)*q ; T3 = s3_b*g ; z1 -= T3 ; bot = z1-g
                s3 = t(3, "s3")
                V.scalar_tensor_tensor(s3, p2, KS, qq, MUL, MUL)
                t3t = t(21, "t3t")
                t3t4 = t3t.rearrange("p (a j f) -> p a j f", a=3, j=7, f=F)
                s3_b = (
                    s3.rearrange("p (a f) -> p a f", a=3, f=F)
                    .unsqueeze(2)
                    .broadcast_to([P, 3, 7, F])
                )
                V.tensor_mul(t3t4, s3_b, g4)
                V.tensor_sub(z1, z1, t3t)
                V.tensor_sub(bot_f, z1, g)

                # top = (r x bot) - jtw   (r = rE/RS)
                rQ = t(3, "rQ")
                rQ3 = rQ.rearrange("p (a f) -> p a f", a=3, f=F)
                V.scalar_tensor_tensor(rQ3, rE3, 1.0 / RS, zero_b, MUL, ADD)
                ctb = t(21, "ctb")
                ctc = t(21, "ctc")
                ctb3 = ctb.rearrange("p (a x) -> p a x", a=3, x=7 * F)
                ctc3 = ctc.rearrange("p (a x) -> p a x", a=3, x=7 * F)
                bot3 = bot_f.rearrange("p (a x) -> p a x", a=3, x=7 * F)
                for a in range(3):
                    a1_, a2_ = (a + 1) % 3, (a + 2) % 3
                    r1 = rQ3[:, a1_ : a1_ + 1, :].broadcast_to([P, 7, F])
                    r2 = rQ3[:, a2_ : a2_ + 1, :].broadcast_to([P, 7, F])
                    b2 = bot3[:, a2_].rearrange("p (j f) -> p j f", j=7, f=F)
                    b1 = bot3[:, a1_].rearrange("p (j f) -> p j f", j=7, f=F)
                    V.tensor_mul(ctb3[:, a].rearrange("p (j f) -> p j f", j=7, f=F), r1, b2)
                    V.tensor_mul(ctc3[:, a].rearrange("p (j f) -> p j f", j=7, f=F), r2, b1)
                gs(ctb, ctb, ctc)
                V.tensor_sub(top_f, ctb, jtw)

                nc.scalar.dma_start(out_d[b], outt[:])

            st_prev = None
            for b in range(NBLK):
                st = front(b)
                if st_prev is not None:
                    back(st_prev, b - 1)
                st_prev = st
            back(st_prev, NBLK - 1)

    nc.compile()
    return nc


_NC_CACHE = None


def _get_nc():
    global _NC_CACHE
    if _NC_CACHE is None:
        _NC_CACHE = build_nc()
    return _NC_CACHE


def _shard_inputs(com_list, link_pose_list, jacobian):
    com = np.asarray(com_list, np.float32).reshape(N_SAMPLES * N_HORIZON, 3, 7)
    lnk = np.asarray(link_pose_list, np.float32).reshape(
        N_SAMPLES * N_HORIZON, 4, 4, 9
    )
    jac = np.asarray(jacobian, np.float32).reshape(N_SAMPLES * N_HORIZON, 6, 7)
    pos = lnk[:, :3, 3, :7]  # (n,3,7)
    j3 = jac[:, :3, :]  # (n,3,7)

    # pack channels: [com 21, pos 21, jac 21] -> fp16 ch-major [NBLK,P,63,F]
    packed = np.concatenate(
        [com.reshape(-1, 21), pos.reshape(-1, 21), j3.reshape(-1, 21)], axis=1
    ).astype(np.float16)  # (n, 63)

    cst = _const_array().reshape(P, NCCH * F)
    in_maps = []
    for c in range(N_CORES):
        blk = packed[c * SPC : (c + 1) * SPC].reshape(NBLK, P, F, 63)
        blk = np.ascontiguousarray(blk.transpose(0, 1, 3, 2))  # [NBLK,P,63,F]
        in_maps.append({"inp": blk.reshape(NBLK, P, 63 * F), "cst": cst})
    return in_maps


def _gather(results):
    outs = []
    for r in results:
        o = np.asarray(r["out"], np.float32).reshape(NBLK, P, 42, F)
        o = o.transpose(0, 1, 3, 2).reshape(SPC, 6, 7)
        outs.append(o)
    full = np.concatenate(outs, axis=0)
    return np.ascontiguousarray(full.reshape(N_SAMPLES, N_HORIZON, 6, 7))


def run(com_list, link_pose_list, jacobian, trace=False):
    nc = _get_nc()
    in_maps = _shard_inputs(com_list, link_pose_list, jacobian)
    res = run_bass_kernel_spmd(nc, in_maps, list(range(N_CORES)), trace=trace)
    return _gather(res.results), res


def kernel(com_list, link_pose_list, jacobian):
    out, _ = run(com_list, link_pose_list, jacobian)
    return out
